# revision 38
# baseline (speedup 1.0000x reference)
"""GATConv (graph attention) kernel for 8 Trainium2 NeuronCores.

Strategy (graph/data parallel, sharded by destination node, bf16 compute):
  Phase 1 (8-way sharded): each core projects its block of node features
      h = feat_blk @ fc_w.T (bf16 in, fp32 accum) and per-node attention
      logits el/er, folded into the same matmul via host-combined weights
      W_comb = [fc_w.T | fc_w.T @ blockdiag(attn_l, attn_r)].  Host feeds
      feat pre-transposed so no on-device transposes are needed.
  Host relay (pure indexing): assemble the full bf16 h table (two int16-
      addressable halves), sort edges by dst window into per-(window,half)
      buckets of uniform capacity CH, expand el[src]/er[dst] per edge slot.
  Phase 2 (the memory-bound part): per core, two contiguous gather
      streams (src<HALF table A, then src>=HALF table B), chunked
      dma_gather of bf16 h rows (256 B each) cycling 4 SWDGE queues;
      ee = exp(leaky(el+er)) in fp32 -> bf16; fat = [h[src]*ee | ee]
      (132 cols); per-128-edge-group one-hot selection matrices
      (is_equal vs iota) feed bf16 PE matmuls that scatter-add messages
      and softmax denominators into a per-window PSUM accumulator;
      A-pass partials parked in SBUF, B-pass combines, normalizes by the
      ee sums, adds bias, and writes out.

out[n] = (sum_e ee_e * h[src_e]) / (sum_e ee_e) + bias   (softmax folded)
"""

import sys

for _p in ("/opt/trn_rl_repo", "/root/.axon_site/_ro/trn_rl_repo"):
    if _p not in sys.path:
        sys.path.append(_p)

from contextlib import ExitStack

import numpy as np
import ml_dtypes

import concourse.bass as bass
import concourse.tile as tile
from concourse import bacc, mybir
from concourse.bass_utils import run_bass_kernel_spmd

F32 = mybir.dt.float32
BF16 = mybir.dt.bfloat16
I16 = mybir.dt.int16
AF = mybir.ActivationFunctionType
OP = mybir.AluOpType
P = 128
GCHUNK = 1024          # dma_gather ucode hard limit per call
SLOTS_PER_BATCH = 4096
BF = ml_dtypes.bfloat16


def _apx(t, offset, pattern):
    """Custom free-dim access pattern into a pool tile."""
    a = t[:]
    return bass.AP(a.tensor, a.offset + offset, [list(a.ap[0])] + pattern)


def _roundup(x, m):
    return (x + m - 1) // m * m


class GATKernel:
    def __init__(self, N, F, H, D, NC, neg_slope=0.2):
        self.N, self.F, self.H, self.D, self.NC = N, F, H, D, NC
        self.HD = H * D
        assert self.HD == P and F % P == 0 and N % NC == 0
        self.KT = F // P
        self.NEG = neg_slope
        self.NB = N // NC                    # nodes per core block
        self.W = (self.NB + P - 1) // P      # windows per core
        self.NBP = self.W * P                # padded block size
        self.HALF = ((N // 2 + 127) // 128) * 128
        self.NPAD = 2 * self.HALF            # padded h table size
        assert self.HALF < 32768 and self.NPAD >= N
        self.CH = None                       # capacity per (window, half)
        self._nc1 = None
        self._nc2 = None
        self._pp = None

    # ---------------- host-side preprocessing (indexing only) -----------

    def _preprocess(self, src, dst):
        N, NB, NBP, W, NC = self.N, self.NB, self.NBP, self.W, self.NC
        HALF = self.HALF
        src = np.asarray(src).astype(np.int64)
        dst = np.asarray(dst).astype(np.int64)
        core_of = dst // NB
        per_core = []
        for c in range(NC):
            em = np.nonzero(core_of == c)[0]
            d_loc = dst[em] - c * NB
            s_glob = src[em]
            isB = s_glob >= HALF
            degA = np.bincount(d_loc[~isB], minlength=NBP)
            degB = np.bincount(d_loc[isB], minlength=NBP)
            # zero-degree nodes (incl. padding slots) get one dummy B edge
            # (gathers a zero row, ee=1) so denominators are never 0.
            dummy = (degA + degB) == 0
            degB = degB + dummy
            # greedy bin-pack nodes into W windows of <= P nodes, balancing
            # the max of per-half loads
            order = np.argsort(-(degA + degB), kind="stable")
            wA = np.zeros(W, np.int64)
            wB = np.zeros(W, np.int64)
            wn = np.zeros(W, np.int64)
            node_win = np.empty(NBP, np.int64)
            node_slot = np.empty(NBP, np.int64)
            big = np.iinfo(np.int64).max
            for n in order:
                score = np.maximum(wA + degA[n], wB + degB[n])
                score = np.where(wn < P, score, big)
                w = int(np.argmin(score))
                node_win[n] = w
                node_slot[n] = wn[w]
                wn[w] += 1
                wA[w] += degA[n]
                wB[w] += degB[n]
            per_core.append(dict(em=em, d_loc=d_loc, s_glob=s_glob, isB=isB,
                                 node_win=node_win, node_slot=node_slot,
                                 dummy=dummy, wA=wA, wB=wB))
        cap = max(max(int(d["wA"].max()), int(d["wB"].max())) for d in per_core)
        CH = _roundup(cap, P)
        self.CH = CH
        SAP = _roundup(W * CH, GCHUNK)       # padded half-stream length
        self.SAP = SAP
        GT = SAP // P                        # groups per half incl pad

        for c, d in enumerate(per_core):
            gidx = np.zeros(2 * SAP, np.int16)
            elid = np.full(2 * SAP, -1, np.int64)
            erid = np.full(2 * SAP, -1, np.int64)
            slotv = np.zeros(2 * SAP, np.int64)
            for half in (0, 1):
                if half == 0:
                    eids = np.nonzero(~d["isB"])[0]
                    nd = d["d_loc"][eids]
                    gi = d["s_glob"][eids]
                    el = d["s_glob"][eids]
                else:
                    eids = np.nonzero(d["isB"])[0]
                    dn = np.nonzero(d["dummy"])[0]
                    nd = np.concatenate([d["d_loc"][eids], dn])
                    gi = np.concatenate([d["s_glob"][eids] - HALF,
                                         np.full(len(dn), self.N - HALF)])
                    el = np.concatenate([d["s_glob"][eids],
                                         np.full(len(dn), -2)])
                win = d["node_win"][nd]
                # within-bucket ascending src: gather descriptors hit HBM
                # rows in order, improving drain locality
                order = np.lexsort((gi, win))
                ws = win[order]
                newb = np.r_[True, ws[1:] != ws[:-1]]
                firstidx = np.nonzero(newb)[0]
                runlen = np.diff(np.r_[firstidx, len(ws)])
                off = np.arange(len(ws)) - np.repeat(firstidx, runlen)
                pos = half * SAP + ws * CH + off
                assert off.max() < CH
                gidx[pos] = gi[order].astype(np.int16)
                elid[pos] = el[order]
                er = nd + c * NB
                er[nd >= NB] = -1            # block-padding nodes: er = 0
                erid[pos] = er[order]
                slotv[pos] = d["node_slot"][nd][order]

            d["gidx_w"] = np.ascontiguousarray(
                np.tile(gidx.reshape(2 * SAP // 16, 16).T, (8, 1)))
            d["slot_w"] = np.ascontiguousarray(
                slotv.reshape(2 * GT, P).T.astype(BF))
            d["elid"] = elid
            d["erid"] = erid
            d["out_row"] = (d["node_win"][:NB] * P + d["node_slot"][:NB])
        self._pp = per_core
        return per_core

    # ---------------- phase 1: projection + logits ----------------------

    def _build_phase1(self):
        F, H, HD, KT, W = self.F, self.H, self.HD, self.KT, self.W
        NW = HD + 2 * H
        nc = bacc.Bacc("TRN2", target_bir_lowering=False, debug=False,
                       num_devices=self.NC)
        # all layouts partition-major and contiguous: one big DMA each way
        featd = nc.dram_tensor("featT", [P, W, KT, P], BF16,
                               kind="ExternalInput")
        wd = nc.dram_tensor("wcomb", [P, KT, NW], BF16, kind="ExternalInput")
        hd = nc.dram_tensor("h", [P, W, HD], BF16, kind="ExternalOutput")
        elrd = nc.dram_tensor("elr", [P, W, 2 * H], F32,
                              kind="ExternalOutput")

        with tile.TileContext(nc) as tc, ExitStack() as ctx:
            const = ctx.enter_context(tc.tile_pool(name="const", bufs=1))
            psum = ctx.enter_context(tc.tile_pool(name="ps", bufs=4,
                                                  space="PSUM"))

            wt = const.tile([P, KT, NW], BF16)
            nc.sync.dma_start(wt[:], wd.ap()[:, :, :])
            ftall = const.tile([P, W, KT, P], BF16)
            nc.sync.dma_start(ftall[:], featd.ap()[:, :, :, :])
            hbig = const.tile([P, W, HD], BF16)
            ebig = const.tile([P, W, 2 * H], F32)

            for t in range(W):
                pt = psum.tile([P, NW], F32, tag="pt")
                for k in range(KT):
                    nc.tensor.matmul(pt[:], ftall[:, t, k, :], wt[:, k, :],
                                     start=(k == 0), stop=(k == KT - 1))
                nc.vector.tensor_copy(hbig[:, t, :], pt[:, 0:HD])
                nc.scalar.activation(ebig[:, t, :], pt[:, HD:NW], AF.Copy)
            nc.sync.dma_start(hd.ap()[:, :, :], hbig[:])
            nc.sync.dma_start(elrd.ap()[:, :, :], ebig[:])
        nc.compile()
        return nc

    # ---------------- phase 2: gather + segment softmax + aggregate -----

    def _build_phase2(self):
        H, HD, W, NBP, HALF, CH = self.H, self.HD, self.W, self.NBP, \
            self.HALF, self.CH
        G = CH // P
        SAP = self.SAP
        GT = SAP // P
        HDE = HD + H
        nc = bacc.Bacc("TRN2", target_bir_lowering=False, debug=False,
                       num_devices=self.NC, num_swdge_queues=4,
                       dynamic_dma_scratch_size=32768)
        hAd = nc.dram_tensor("hA", [HALF, HD], BF16, kind="ExternalInput")
        hBd = nc.dram_tensor("hB", [HALF + P, HD], BF16, kind="ExternalInput")
        gixd = nc.dram_tensor("gidx", [P, 2 * SAP // 16], I16,
                              kind="ExternalInput")
        elxd = nc.dram_tensor("elx", [P, 2 * GT, H], BF16,
                              kind="ExternalInput")
        erxd = nc.dram_tensor("erx", [P, 2 * GT, H], BF16,
                              kind="ExternalInput")
        slotd = nc.dram_tensor("slot", [P, 2 * GT], BF16,
                               kind="ExternalInput")
        iotad = nc.dram_tensor("iota", [P, P], BF16, kind="ExternalInput")
        biasd = nc.dram_tensor("biast", [P, HD], F32, kind="ExternalInput")
        outd = nc.dram_tensor("outp", [NBP, HD], F32, kind="ExternalOutput")

        with tile.TileContext(nc) as tc, ExitStack() as ctx:
            const = ctx.enter_context(tc.tile_pool(name="const", bufs=1))
            gpool = ctx.enter_context(tc.tile_pool(name="gat", bufs=5))
            fpool = ctx.enter_context(tc.tile_pool(name="fat", bufs=4))
            spool = ctx.enter_context(tc.tile_pool(name="side", bufs=3))
            epool = ctx.enter_context(tc.tile_pool(name="ee3", bufs=3))
            selp = ctx.enter_context(tc.tile_pool(name="sel", bufs=3))
            psum = ctx.enter_context(tc.tile_pool(name="acc", bufs=4,
                                                  space="PSUM"))
            opool = ctx.enter_context(tc.tile_pool(name="out", bufs=3))

            iot = const.tile([P, P], BF16)
            nc.sync.dma_start(iot[:], iotad.ap()[:, :])
            bia = const.tile([P, HD], F32)
            nc.sync.dma_start(bia[:], biasd.ap()[:, :])
            partA = const.tile([P, W, HDE], F32)
            # whole index stream resident: gathers never wait on idx DMAs.
            # Split the load so the first gathers aren't gated on the tail.
            idxall = const.tile([P, 2 * SAP // 16], I16)
            icut = SLOTS_PER_BATCH // 16
            nc.sync.dma_start(idxall[:, 0:icut], gixd.ap()[:, 0:icut])
            nc.sync.dma_start(idxall[:, icut:2 * SAP // 16],
                              gixd.ap()[:, icut:2 * SAP // 16])
            # slot stream fully resident: sel gen decouples from batch tiles
            slotall = const.tile([P, 2 * GT], BF16)
            nc.sync.dma_start(slotall[:], slotd.ap()[:, :])

            qn = 0
            for half, tabd in ((0, hAd), (1, hBd)):
                # fat tiles covering the half's groups, in emission order;
                # fatinfo[i] = (tile, first group, ngroups)
                fatinfo = []
                w_done = 0

                def fat_ap(g, cols, width):
                    """AP over group g's [P, width] slice at col offset."""
                    for t, g0, ng in fatinfo:
                        if g0 <= g < g0 + ng:
                            return _apx(t, (g - g0) * HDE + cols,
                                        [[1, width]])
                    raise AssertionError

                for s0 in range(0, SAP, SLOTS_PER_BATCH):
                    s1 = min(SAP, s0 + SLOTS_PER_BATCH)
                    nslots = s1 - s0
                    ngr = nslots // P
                    ncalls = nslots // GCHUNK
                    g0 = s0 // P                  # group offset within half

                    buf = gpool.tile([P, ngr, HD], BF16, tag="buf")
                    for ci in range(ncalls):
                        # clip trailing pad slots (beyond W*CH): never
                        # scattered, so don't pay Q7 descriptors for them
                        nreal = min(GCHUNK, W * CH - (s0 + ci * GCHUNK))
                        if nreal <= 0:
                            continue
                        ncall = _roundup(nreal, P)
                        ob = _apx(buf, ci * 8 * HD,
                                  [[HD, ncall // P], [1, HD]])
                        oi = _apx(idxall,
                                  (half * SAP + s0 + ci * GCHUNK) // 16,
                                  [[1, ncall // 16]])
                        nc.gpsimd.dma_gather(ob, tabd.ap()[:, :], oi,
                                             ncall, ncall, HD,
                                             queue_num=qn % 4)
                        qn += 1

                    gg = half * GT + g0
                    elt = spool.tile([P, ngr, H], BF16, tag="elt")
                    nc.sync.dma_start(elt[:], elxd.ap()[:, gg:gg + ngr, :])
                    ert = spool.tile([P, ngr, H], BF16, tag="ert")
                    nc.sync.dma_start(ert[:], erxd.ap()[:, gg:gg + ngr, :])
                    # ee = exp(max(t, NEG*t)), t = el + er  (flat APs)
                    nh = ngr * H
                    tt = spool.tile([P, ngr, H], F32, tag="tt")
                    nc.vector.tensor_add(_apx(tt, 0, [[1, nh]]),
                                         _apx(elt, 0, [[1, nh]]),
                                         _apx(ert, 0, [[1, nh]]))
                    # leaky = (t * NEG) max t in one pass
                    nc.vector.scalar_tensor_tensor(
                        _apx(tt, 0, [[1, nh]]), _apx(tt, 0, [[1, nh]]),
                        self.NEG, _apx(tt, 0, [[1, nh]]),
                        OP.mult, OP.max)
                    ee = spool.tile([P, ngr, H], BF16, tag="ee")
                    nc.scalar.activation(_apx(ee, 0, [[1, nh]]),
                                         _apx(tt, 0, [[1, nh]]), AF.Exp)

                    # ee expanded to 128 cols (ACT), then contiguous DVE mul
                    ee3 = epool.tile([P, ngr, HD], BF16, tag="ee3")
                    fat = fpool.tile([P, ngr, HDE], BF16, tag="fat")
                    fatinfo.append((fat, g0, ngr))
                    for ci in range(ncalls):
                        go = ci * 8
                        e3o = _apx(ee3, go * HD, [[HD, 8], [32, H], [1, 32]])
                        eei = _apx(ee, go * H, [[H, 8], [1, H], [0, 32]])
                        nc.scalar.activation(e3o, eei, AF.Copy)
                        outm = _apx(fat, go * HDE, [[HDE, 8], [1, HD]])
                        inm = _apx(buf, go * HD, [[HD, 8], [1, HD]])
                        e3i = _apx(ee3, go * HD, [[HD, 8], [1, HD]])
                        nc.vector.tensor_mul(outm, inm, e3i)
                    oute = _apx(fat, HD, [[HDE, ngr], [1, H]])
                    eein = _apx(ee, 0, [[H, ngr], [1, H]])
                    nc.scalar.activation(oute, eein, AF.Copy)

                    # scatter windows fully covered by the stream so far
                    w_avail = min(W, (s1 // P) // G)
                    for w in range(w_done, w_avail):
                        sel = selp.tile([P, G, P], BF16, tag="sel")
                        slto = _apx(slotall, half * GT + w * G,
                                    [[1, G], [0, P]])
                        ioto = _apx(iot, 0, [[0, G], [1, P]])
                        nc.vector.tensor_tensor(sel[:], slto, ioto,
                                                OP.is_equal)
                        ps = psum.tile([P, HDE], F32, tag="ps")
                        for k in range(G):
                            nc.tensor.matmul(ps[:], sel[:, k, :],
                                             fat_ap(w * G + k, 0, HDE),
                                             start=(k == 0),
                                             stop=(k == G - 1))
                        if half == 0:
                            nc.scalar.activation(partA[:, w, :], ps[:],
                                                 AF.Copy)
                        else:
                            den = opool.tile([P, H], F32, tag="den")
                            nc.vector.tensor_add(
                                den[:], ps[:, HD:HDE], partA[:, w, HD:HDE])
                            rec = opool.tile([P, H], F32, tag="rec")
                            nc.vector.reciprocal(rec[:], den[:])
                            ms = opool.tile([P, HD], F32, tag="ms")
                            nc.vector.tensor_add(
                                ms[:], ps[:, 0:HD], partA[:, w, 0:HD])
                            ot = opool.tile([P, HD], F32, tag="ot")
                            oto = _apx(ot, 0, [[32, H], [1, 32]])
                            mso = _apx(ms, 0, [[32, H], [1, 32]])
                            reco = _apx(rec, 0, [[1, H], [0, 32]])
                            nc.vector.tensor_tensor(oto, reco, mso, OP.mult)
                            nc.vector.tensor_add(ot[:], ot[:], bia[:])
                            nc.sync.dma_start(
                                outd.ap()[w * P:(w + 1) * P, :], ot[:])
                    w_done = w_avail
        nc.compile()
        return nc

    # ---------------- orchestration -------------------------------------

    def run(self, feat, fc_w, attn_l, attn_r, bias, src, dst, trace=False):
        N, F, H, D, NC = self.N, self.F, self.H, self.D, self.NC
        NB, NBP, HD, HALF, NPAD = self.NB, self.NBP, self.HD, self.HALF, \
            self.NPAD
        KT, W = self.KT, self.W
        feat = np.ascontiguousarray(np.asarray(feat, np.float32))
        fc_w = np.ascontiguousarray(np.asarray(fc_w, np.float32))
        attn_l = np.asarray(attn_l, np.float32)
        attn_r = np.asarray(attn_r, np.float32)
        bias = np.asarray(bias, np.float32)

        fp = (np.asarray(src)[:64].tobytes(), np.asarray(dst)[:64].tobytes(),
              len(np.asarray(src)))
        if self._pp is None or getattr(self, "_fp", None) != fp:
            old = (self.CH, getattr(self, "SAP", None))
            self._preprocess(src, dst)
            self._fp = fp
            if old != (self.CH, self.SAP):
                self._nc2 = None   # capacity changed; rebuild phase 2
        pp = self._pp
        if self._nc1 is None:
            self._nc1 = self._build_phase1()
        if self._nc2 is None:
            self._nc2 = self._build_phase2()

        # combined projection + logit weights: [F, HD | 2H]
        ablk = np.zeros((HD, 2 * H), np.float32)
        for h in range(H):
            ablk[h * D:(h + 1) * D, h] = attn_l[h]
            ablk[h * D:(h + 1) * D, H + h] = attn_r[h]
        wcomb = np.concatenate([fc_w.T, fc_w.T @ ablk], axis=1)  # [F, NW]
        # [P(f%128), KT, NW] partition-major
        wcomb = np.ascontiguousarray(
            wcomb.reshape(KT, P, HD + 2 * H).transpose(1, 0, 2).astype(BF))

        in1 = []
        for c in range(NC):
            fb = np.zeros((NBP, F), np.float32)
            fb[:NB] = feat[c * NB:(c + 1) * NB]
            # [P(f'), W, KT, P(n)]: window t, k-chunk, node n -> partition f'
            fT = np.ascontiguousarray(
                fb.reshape(W, P, KT, P).transpose(3, 0, 2, 1).astype(BF))
            in1.append({"featT": fT, "wcomb": wcomb})
        r1 = run_bass_kernel_spmd(self._nc1, in1, list(range(NC)),
                                  trace=trace)
        t1 = r1.exec_time_ns
        self.t1 = t1
        self.trace1 = (r1.instructions_and_trace[1]
                       if r1.instructions_and_trace else None)

        h_full = np.zeros((NPAD, HD), BF)
        el_full = np.zeros((N, H), np.float32)
        er_full = np.zeros((N, H), np.float32)
        for c in range(NC):
            hblk = r1.results[c]["h"].transpose(1, 0, 2).reshape(NBP, HD)
            h_full[c * NB:(c + 1) * NB] = hblk[:NB]
            elr = r1.results[c]["elr"].transpose(1, 0, 2).reshape(
                NBP, 2 * H)[:NB]
            el_full[c * NB:(c + 1) * NB] = elr[:, :H]
            er_full[c * NB:(c + 1) * NB] = elr[:, H:]

        hA = np.ascontiguousarray(h_full[:HALF])
        hB = np.concatenate(
            [h_full[HALF:], np.zeros((P, HD), BF)]).astype(BF)
        hB = np.ascontiguousarray(hB)
        iota = np.tile(np.arange(P, dtype=np.float32), (P, 1)).astype(BF)
        biast = np.tile(bias.reshape(1, HD), (P, 1)).astype(np.float32)

        SAP = self.SAP
        GT = SAP // P
        in2 = []
        for c in range(NC):
            d = pp[c]
            elid, erid = d["elid"], d["erid"]
            # elid: >=0 real, -1 padding (ee=0), -2 dummy (ee=1)
            elx = np.zeros((2 * SAP, H), np.float32)
            real = elid >= 0
            elx[real] = el_full[elid[real]]
            elx[elid == -1] = -1e30
            erx = np.zeros((2 * SAP, H), np.float32)
            rer = erid >= 0
            erx[rer] = er_full[erid[rer]]
            in2.append({
                "hA": hA, "hB": hB,
                "gidx": d["gidx_w"],
                "elx": np.ascontiguousarray(
                    elx.reshape(2 * GT, P, H).transpose(1, 0, 2)
                    .astype(BF)),
                "erx": np.ascontiguousarray(
                    erx.reshape(2 * GT, P, H).transpose(1, 0, 2)
                    .astype(BF)),
                "slot": d["slot_w"],
                "iota": iota, "biast": biast,
            })
        r2 = run_bass_kernel_spmd(self._nc2, in2, list(range(NC)),
                                  trace=trace)
        t2 = r2.exec_time_ns
        self.t2 = t2
        self.trace2 = (r2.instructions_and_trace[1]
                       if r2.instructions_and_trace else None)

        out = np.empty((N, HD), np.float32)
        for c in range(NC):
            blk = r2.results[c]["outp"]
            out[c * NB:(c + 1) * NB] = blk[pp[c]["out_row"]]
        self.exec_ns = ((t1 or 0) + (t2 or 0)) or None
        return out.reshape(N, H, D)


def _ensure_trace_hook():
    """Register the axon NTFF profile hook if the image's antenv lacks it.

    run_bass_kernel_spmd(trace=True) needs antenv.axon_hooks; some images
    ship antenv without it.  Build the same ctypes hook trn_agent_boot
    would have registered so tracing works when requested (and timing can
    be measured); harmless no-op if already present or impossible.
    """
    import types
    try:
        if "/root/.axon_site" not in sys.path:
            sys.path.append("/root/.axon_site")
        import antenv
        try:
            import antenv.axon_hooks  # noqa: F401
            return
        except ImportError:
            pass
        from trn_agent_boot.trn_boot import _ntff_profile_via_ctypes
        hook = [_ntff_profile_via_ctypes("/opt/axon/libaxon_pjrt.so")]
        mod = types.ModuleType("antenv.axon_hooks")
        mod.set_axon_ntff_profile_hook = lambda h: hook.__setitem__(0, h)
        mod.get_axon_ntff_profile_hook = lambda: hook[0]
        sys.modules["antenv.axon_hooks"] = mod
        antenv.axon_hooks = mod
    except Exception:
        pass


_CACHED = None


def kernel(feat, fc_w, attn_l, attn_r, bias, src, dst):
    global _CACHED
    if _CACHED is None:
        _CACHED = GATKernel(N=50000, F=256, H=4, D=32, NC=8)
    import os
    tr = bool(int(os.environ.get("GAT_TRACE", "0")))
    if tr or os.environ.get("BASS_TRACE"):
        _ensure_trace_hook()
    return _CACHED.run(feat, fc_w, attn_l, attn_r, bias, src, dst, trace=tr)
